# revision 1
# baseline (speedup 1.0000x reference)
"""Trainium2 Bass kernel for nn_HadamardTransform: Y = X @ H4096_normalized.

Algorithm: H4096 (Sylvester, normalized) factors exactly as the Kronecker
product H16n (x) H256n over the column index c = 256*i + j (i in 16,
j in 256).  Each row x of X, reshaped to R[16, 256], transforms as
Y_mat = G @ R @ H256u with G = 2^-6 * H16u (all of the 2^-6
normalization folded into the 16-side so H256u stays exactly +-1).

Per 32-row group (SBUF tile [128, 1024]; partition p = 16*b + i with
b in 8 rows, free f = 256*a + j with a in 4; row r = 32*g + 8*a + b):
  MM-A (per a, jh-half):   pa[j_sub, (b,i')] = xb_slice.T @ W1
       W1 = I8 (x) G, block-diagonal 128x128 -> the i-transform,
       j emerging on partitions (no transposes needed anywhere).
  MM-B (per a, accumulating jh): pb[(b,i'), j'] += sa_jh.T @ HJ[jh]
       HJ = H256u split into two 128-row halves -> the j-transform,
       natural output layout.

All matmuls run in bf16 (1 PE cycle/row vs 4 for fp32; W1/HJ entries
are +-2^-6 / +-1, exactly representable in bf16).  X is loaded through
SWDGE cast-DMA (fp32 HBM read -> bf16 SBUF write, the convert rides the
SDMA datapath, so no compute-engine op and half the SBUF write traffic).
PSUM->SBUF copies: stage A on DVE (bf16 out), stage B on ACT (bf16 out).
Y is stored as bf16 (rel err ~3e-3, tolerance is 2e-2) and upcast to
fp32 on the host, cutting store traffic in half.  Loads ride the
gpsimd/SWDGE queue, stores the ACT HWDGE ring.

Sharding: X's 8192 rows split into 8 contiguous shards of 1024 rows,
one per NeuronCore (pure data parallelism, no collectives).

Measured: ~75.4 us HW exec (vs 151.9 us fp32 baseline), rel err 2.9e-3.
"""

import sys

import numpy as np

try:
    import concourse.bass as bass
except ImportError:
    sys.path.insert(0, "/opt/trn_rl_repo")
    import concourse.bass as bass

import concourse.mybir as mybir
import concourse.tile as tile
from concourse import bacc
from concourse.bass_utils import run_bass_kernel_spmd

N_CORES = 8
ROWS = 8192
N = 4096
ROWS_PER_CORE = ROWS // N_CORES  # 1024
ROWS_PER_GROUP = 32
GROUPS = ROWS_PER_CORE // ROWS_PER_GROUP  # 32
F32 = mybir.dt.float32
BF16 = mybir.dt.bfloat16
NP_BF16 = mybir.dt.np(BF16)

NI = 16   # i-side order
NJ = 256  # j-side order
NB = 128 // NI  # 8 rows per partition block
NA = 1024 // NJ  # 4 free-dim row blocks


def _hadamard_u(n: int) -> np.ndarray:
    H = np.array([[1.0]], dtype=np.float64)
    while H.shape[0] < n:
        H = np.block([[H, H], [H, -H]])
    return H


def _constants() -> tuple[np.ndarray, np.ndarray]:
    G = (2.0 ** -6) * _hadamard_u(NI)
    W1 = np.kron(np.eye(NB), G).astype(NP_BF16)     # [128,128] block-diag
    HJ = _hadamard_u(NJ).astype(NP_BF16)            # [256,256] exact +-1
    return W1, HJ


def _build_bass(loop_reps: int | None = None):
    nc = bacc.Bacc("TRN2", target_bir_lowering=False, debug=False)

    X = nc.dram_tensor("X", [ROWS_PER_CORE, N], F32, kind="ExternalInput")
    W1 = nc.dram_tensor("W1", [128, 128], BF16, kind="ExternalInput")
    HJ = nc.dram_tensor("HJ", [NJ, NJ], BF16, kind="ExternalInput")
    Y = nc.dram_tensor("Y", [ROWS_PER_CORE, N], BF16, kind="ExternalOutput")

    # row r = 32*g + 8*a + b ; column c = 256*i + j
    # SBUF group tile: partition p = 16*b + i, free f = 256*a + j
    X_re = X[:].rearrange(
        "(g a b) (i j) -> g b i a j", a=NA, b=NB, i=NI, j=NJ
    )
    Y_re = Y[:].rearrange(
        "(g a b) (i j) -> g b i a j", a=NA, b=NB, i=NI, j=NJ
    )

    with tile.TileContext(nc) as tc:
        with (
            tc.tile_pool(name="consts", bufs=1) as cpool,
            tc.tile_pool(name="xbf", bufs=6) as xbpool,
            tc.tile_pool(name="yout", bufs=4) as ypool,
            tc.tile_pool(name="mid", bufs=6) as spool,
            tc.tile_pool(name="psA", bufs=4, space="PSUM") as psA,
            tc.tile_pool(name="psB", bufs=4, space="PSUM") as psB,
        ):
            w1 = cpool.tile([128, 128], BF16)
            nc.sync.dma_start(out=w1[:], in_=W1[:])
            # hj[:, jh*NJ:(jh+1)*NJ] = rows [128*jh, 128*(jh+1)) of HJ
            hj = cpool.tile([128, 2 * NJ], BF16)
            hj_3d = hj[:].rearrange("p (jh n) -> p jh n", jh=2, n=NJ)
            nc.sync.dma_start(
                out=hj_3d, in_=HJ[:].rearrange("(jh k) n -> k jh n", jh=2)
            )

            def flush_b(state):
                """MM-B x8 (accumulating jh pairs) + 2 ACT copies + store
                for a previously A-staged group."""
                if state is None:
                    return
                sa01, yw_3d_, yw_, g_ = state
                for half in range(2):           # a in {0,1} then {2,3}
                    pb = psB.tile([128, 512], F32)
                    for aa in range(2):
                        a = 2 * half + aa
                        for jh in range(2):
                            nc.tensor.matmul(
                                pb[:, aa * NJ:(aa + 1) * NJ],
                                lhsT=sa01[jh][:, a * 128:(a + 1) * 128],
                                rhs=hj[:, jh * NJ:(jh + 1) * NJ],
                                start=(jh == 0),
                                stop=(jh == 1),
                            )
                    nc.scalar.copy(
                        out=yw_[:, half * 512:(half + 1) * 512], in_=pb[:]
                    )
                nc.scalar.dma_start(out=Y_re[g_], in_=yw_3d_)

            def emit_body():
              # 1-group software pipeline: group g's B-stage is emitted
              # after group g+1's A-stage (B needs both jh halves of sa).
              prev = None
              for g in range(GROUPS):
                xb = xbpool.tile([128, 1024], BF16)
                xb_3d = xb[:].rearrange("p (a j) -> p a j", a=NA, j=NJ)
                # SWDGE cast-DMA: fp32 HBM read -> bf16 SBUF write; the
                # fp32->bf16 convert rides the SDMA datapath (no engine op).
                nc.gpsimd.dma_start(out=xb_3d, in_=X_re[g])
                yw = ypool.tile([128, 1024], BF16)
                yw_3d = yw[:].rearrange("p (a j) -> p a j", a=NA, j=NJ)
                sa01 = []
                for jh in range(2):
                    pa = psA.tile([128, 512], F32)
                    for a in range(4):
                        nc.tensor.matmul(
                            pa[:, a * 128:(a + 1) * 128],
                            lhsT=xb[:, a * NJ + jh * 128:
                                       a * NJ + jh * 128 + 128],
                            rhs=w1[:],
                            start=True,
                            stop=True,
                        )
                    sa = spool.tile([128, 512], BF16)
                    nc.vector.tensor_copy(out=sa[:], in_=pa[:])
                    sa01.append(sa)
                flush_b(prev)
                prev = (sa01, yw_3d, yw, g)
              flush_b(prev)

            if loop_reps is None:
                emit_body()
            else:
                with tc.For_i(0, loop_reps, 1):
                    emit_body()

    nc.compile()
    return nc


_NC = None


def _get_nc():
    global _NC
    if _NC is None:
        _NC = _build_bass()
    return _NC


def make_in_maps(X: np.ndarray) -> list[dict]:
    W1, HJ = _constants()
    return [
        {
            "X": X[c * ROWS_PER_CORE:(c + 1) * ROWS_PER_CORE],
            "W1": W1,
            "HJ": HJ,
        }
        for c in range(N_CORES)
    ]


def run(X: np.ndarray, trace: bool = False):
    X = np.ascontiguousarray(np.asarray(X, dtype=np.float32))
    assert X.shape == (ROWS, N), X.shape
    nc = _get_nc()
    in_maps = make_in_maps(X)
    res = run_bass_kernel_spmd(
        nc, in_maps, list(range(N_CORES)), trace=trace
    )
    Y = np.concatenate(
        [res.results[c]["Y"].astype(np.float32) for c in range(N_CORES)],
        axis=0,
    )
    return Y, res


def kernel(X, H=None, **_unused) -> np.ndarray:
    Y, _ = run(X, trace=False)
    return Y

